# revision 1
# baseline (speedup 1.0000x reference)
"""AttentionResidualGRU fused Trainium2 kernel.

Strategy: pure data parallelism over batch (8 cores x 32 rows). Per core,
both sequential scans (GRU + attention-correction) run fused in a single
1025-iteration instruction stream.

Per-step GRU matmul is batch-stationary: lhsT = h^T feature-major tiles
[128, 32] (+ a tiny [v0, v1, 1, v0_prev, v1_prev] tile), rhs = host-side
pre-transposed/augmented weights, accumulated in PSUM as
  [ r(512) | z(512) | C=ghn+bhh_n(512) | D=gxn+bih_n(512) | E(4) ]
E = [res0', -res1', v0_prev, v1_prev + res1'] with res' = Wr h + br, which
lets the scan-2 update collapse to x' = x + (E[0:2] * aw0 + E[2:4]) where
aw0 = sigmoid(wd . relu(Wa1 att + ba1) + (ba2[0]-ba2[1]))  [softmax over 2
outputs == sigmoid of the logit difference].

All matmuls run as float32r (full PE rate at N>=256). Biases enter through
the ones-row of the small stationary tile. h^T tiles for the next step are
rebuilt each step with 4 PE transposes into one PSUM bank + 1 copy.
"""

import os
import sys

import numpy as np
import ml_dtypes

BF16 = ml_dtypes.bfloat16

for _p in ("/opt/trn_rl_repo", "/root/.axon_site/_ro/trn_rl_repo"):
    if os.path.isdir(_p) and _p not in sys.path:
        sys.path.append(_p)

B, T, H, IN, OUT = 256, 1024, 512, 2, 2
NCORES = 8
Bc = B // NCORES          # 32
HH = H // 2               # 256
NE = 4
NCOL_H = 1536 + NE        # h-tile rhs cols: r|z|C|E
NCOL_4 = 2048 + NE        # k4 rhs cols:     r|z|C|D|E
SV_COLS = Bc * (T + 1)

_PROG_CACHE = {}


# ----------------------------------------------------------------- host prep

def _prep_consts(W_ih, W_hh, b_ih, b_hh, Wa1, ba1, Wa2, ba2, Wr, br):
    f = np.float32
    W_ih = np.asarray(W_ih, f); W_hh = np.asarray(W_hh, f)
    b_ih = np.asarray(b_ih, f); b_hh = np.asarray(b_hh, f)
    Wa1 = np.asarray(Wa1, f); ba1 = np.asarray(ba1, f)
    Wa2 = np.asarray(Wa2, f); ba2 = np.asarray(ba2, f)
    Wr = np.asarray(Wr, f); br = np.asarray(br, f)

    wh = np.zeros((512, NCOL_H), f)
    for j in range(4):
        fsl = slice(128 * j, 128 * (j + 1))
        m = wh[fsl]
        m[:, 0:512] = W_hh[0:512, fsl].T
        m[:, 512:1024] = W_hh[512:1024, fsl].T
        m[:, 1024:1536] = W_hh[1024:1536, fsl].T
        m[:, 1536] = Wr[0, fsl]
        m[:, 1537] = -Wr[1, fsl]
        m[:, 1539] = Wr[1, fsl]

    # stationary rows are ordered [1, v0_prev, v1_prev, v0, v1]
    w4 = np.zeros((5, NCOL_4), f)
    w4[3:5, 0:512] = W_ih[0:512, :].T
    w4[0, 0:512] = b_ih[0:512] + b_hh[0:512]
    w4[3:5, 512:1024] = W_ih[512:1024, :].T
    w4[0, 512:1024] = b_ih[512:1024] + b_hh[512:1024]
    w4[0, 1024:1536] = b_hh[1024:1536]
    w4[3:5, 1536:2048] = W_ih[1024:1536, :].T
    w4[0, 1536:2048] = b_ih[1024:1536]
    w4[0, 2048] = br[0]
    w4[0, 2049] = -br[1]
    w4[1, 2050] = 1.0
    w4[2, 2051] = 1.0
    w4[0, 2051] = br[1]

    wax = Wa1[:, 0:2].T.copy()   # [2, HH]
    wav = np.zeros((3, HH), f)   # rows [1, v0p, v1p]
    wav[0] = ba1
    wav[1] = Wa1[:, 2]
    wav[2] = Wa1[:, 3]

    wdb = np.broadcast_to((Wa2[0] - Wa2[1]).astype(f), (Bc, HH)).astype(BF16)
    dbias = np.full((Bc, 1), float(ba2[0] - ba2[1]), f)
    ident = np.eye(32, dtype=f).astype(BF16)
    return dict(wh=wh.astype(BF16), w4=w4.astype(BF16), wax=wax.astype(BF16),
                wav=wav.astype(BF16), wdb=wdb, dbias=dbias, ident=ident)


def _prep_core(c, X0, V):
    f = np.float32
    bs = slice(Bc * c, Bc * (c + 1))
    Vc = np.asarray(V[bs], f)                      # [32, T, 2]
    sv5 = np.zeros((5, SV_COLS), f)    # rows [1, v0p, v1p, v0, v1]
    v0 = Vc[:, :, 0].T.reshape(-1)                 # [T*32] time-major
    v1 = Vc[:, :, 1].T.reshape(-1)
    sv5[0, :] = 1.0
    sv5[1, Bc:] = v0
    sv5[2, Bc:] = v1
    sv5[3, : Bc * T] = v0
    sv5[4, : Bc * T] = v1
    x0s = np.zeros((Bc, 32), f)
    x0s[:, 0:2] = np.asarray(X0[bs], f)            # xstage init (cols 0-1 = x)
    x0t = np.zeros((Bc, 32), f)
    x0t[0:2, :] = np.asarray(X0[bs], f).T          # xtb init (rows 0-1 = x^T)
    return dict(sv5=sv5.astype(BF16), x0s=x0s, x0t=x0t.astype(BF16))


# ------------------------------------------------------------- device program

def _build_program(t_steps):
    from concourse import bacc, tile, mybir  # noqa

    f32 = mybir.dt.float32
    bf16 = mybir.dt.bfloat16
    AF = mybir.ActivationFunctionType
    OP = mybir.AluOpType

    nc = bacc.Bacc(None)
    d_wh = nc.declare_dram_parameter("wh", [512, NCOL_H], bf16, isOutput=False)
    d_w4 = nc.declare_dram_parameter("w4", [5, NCOL_4], bf16, isOutput=False)
    d_wax = nc.declare_dram_parameter("wax", [2, HH], bf16, isOutput=False)
    d_wav = nc.declare_dram_parameter("wav", [3, HH], bf16, isOutput=False)
    d_wdb = nc.declare_dram_parameter("wdb", [Bc, HH], bf16, isOutput=False)
    d_dbias = nc.declare_dram_parameter("dbias", [Bc, 1], f32, isOutput=False)
    d_ident = nc.declare_dram_parameter("ident", [32, 32], bf16, isOutput=False)
    d_sv5 = nc.declare_dram_parameter("sv5", [5, SV_COLS], bf16, isOutput=False)
    d_x0s = nc.declare_dram_parameter("x0s", [Bc, 32], f32, isOutput=False)
    d_x0t = nc.declare_dram_parameter("x0t", [Bc, 32], bf16, isOutput=False)
    d_out = nc.declare_dram_parameter("out", [Bc, 2 * T], f32, isOutput=True)

    with tile.TileContext(nc) as tc:
        with (
            tc.tile_pool(name="const", bufs=1) as cpool,
            tc.tile_pool(name="state", bufs=1) as spool,
            tc.tile_pool(name="work", bufs=2) as wpool,
            tc.tile_pool(name="p_rz", bufs=1, space="PSUM") as p_rz,
            tc.tile_pool(name="p_c", bufs=1, space="PSUM") as p_c,
            tc.tile_pool(name="p_d", bufs=1, space="PSUM") as p_d,
            tc.tile_pool(name="p_e", bufs=2, space="PSUM") as p_e,
            tc.tile_pool(name="p_t", bufs=1, space="PSUM") as p_t,
            tc.tile_pool(name="p_f", bufs=1, space="PSUM") as p_f,
        ):
            # constants
            wh_t = [cpool.tile([128, NCOL_H], bf16, tag=f"wh{j}",
                               name=f"wh{j}")
                    for j in range(4)]
            w4_t = cpool.tile([5, NCOL_4], bf16, tag="w4")
            wax_t = cpool.tile([2, HH], bf16, tag="wax")
            wav_t = cpool.tile([3, HH], bf16, tag="wav")
            wdb_t = cpool.tile([Bc, HH], bf16, tag="wdb")
            dbias_t = cpool.tile([Bc, 1], f32, tag="dbias")
            id_t = cpool.tile([32, 32], bf16, tag="ident")
            sv8 = cpool.tile([35, SV_COLS], bf16, tag="sv8")
            for j in range(4):
                nc.sync.dma_start(out=wh_t[j][:],
                                  in_=d_wh[128 * j:128 * (j + 1), :])
            nc.sync.dma_start(out=w4_t[:], in_=d_w4[:])
            nc.sync.dma_start(out=wax_t[:], in_=d_wax[:])
            nc.sync.dma_start(out=wav_t[:], in_=d_wav[:])
            nc.sync.dma_start(out=wdb_t[:], in_=d_wdb[:])
            nc.sync.dma_start(out=dbias_t[:], in_=d_dbias[:])
            nc.sync.dma_start(out=id_t[:], in_=d_ident[:])
            nc.sync.dma_start(out=sv8[0:5, :], in_=d_sv5[:])

            # state
            h_tiles = spool.tile([128, 128], bf16, tag="h_tiles")
            h_bm = spool.tile([Bc, H], bf16, tag="h_bm")
            xstage = spool.tile([Bc, 32], f32, tag="xstage")
            xtb = spool.tile([Bc, 32], bf16, tag="xtb")
            xtf = spool.tile([Bc, 32], f32, tag="xtf")
            out_buf = spool.tile([Bc, 2 * T], f32, tag="out_buf")
            nc.vector.memset(h_bm[:], 0.0)
            nc.sync.dma_start(out=xstage[:], in_=d_x0s[:])
            nc.sync.dma_start(out=xtb[:], in_=d_x0t[:])

            mm = nc.tensor.matmul

            def transposes(hin):
                # h^T stationary tiles for the next h-matmuls
                Tt = p_t.tile([128, 128], bf16, tag="Tt")
                for j in range(4):
                    nc.tensor.transpose(Tt[:, 32 * j: 32 * (j + 1)],
                                        hin[:, 128 * j: 128 * (j + 1)],
                                        id_t[:])
                nc.vector.tensor_copy(h_tiles[:], Tt[:])

            for s in range(t_steps + 1):
                co = Bc * s
                sl4 = sv8[0:5, co: co + Bc]         # [5, 32]
                last = s == t_steps

                # (a) k4 matmuls: independent of h_s -> overlap prev gates
                E = p_e.tile([Bc, NE], f32, tag="E")
                if not last:
                    rz = p_rz.tile([Bc, 1024], f32, tag="rz")
                    C = p_c.tile([Bc, 512], f32, tag="C")
                    D = p_d.tile([Bc, 512], f32, tag="D")
                    mm(rz[:, 0:512], sl4, w4_t[:, 0:512],
                       start=True, stop=(s == 0))
                    mm(rz[:, 512:1024], sl4, w4_t[:, 512:1024],
                       start=True, stop=(s == 0))
                    mm(C[:], sl4, w4_t[:, 1024:1536],
                       start=True, stop=(s == 0))
                    mm(D[:], sl4, w4_t[:, 1536:2048],
                       start=True, stop=True)
                    mm(E[:], sl4, w4_t[:, 2048:2052],
                       start=True, stop=(s == 0))
                else:
                    mm(E[:], sl4, w4_t[:, 2048:2052],
                       start=True, stop=False)

                # (b) h^T tiles from the PREVIOUS step's h (PE queue sits
                # behind (a), so (a) streams during the previous gate tail)
                if s >= 1:
                    transposes(h_bm)

                # (c) h matmuls
                if s >= 1:
                    for j in range(4):
                        st = h_tiles[:, 32 * j: 32 * (j + 1)]
                        fin = j == 3
                        if not last:
                            mm(rz[:, 0:512], st, wh_t[j][:, 0:512],
                               start=False, stop=fin)
                            mm(rz[:, 512:1024], st, wh_t[j][:, 512:1024],
                               start=False, stop=fin)
                            mm(C[:], st, wh_t[j][:, 1024:1536],
                               start=False, stop=fin)
                        mm(E[:], st, wh_t[j][:, 1536:1540],
                           start=False, stop=fin)

                # (d) gates -> h_{s+1} (batch-major, bf16 tail)
                if not last:
                    r_sb = wpool.tile([Bc, 512], bf16, tag="r_sb")
                    z_sb = wpool.tile([Bc, 512], bf16, tag="z_sb")
                    nc.scalar.activation(r_sb[:], rz[:, 0:512], AF.Sigmoid)
                    nc.scalar.activation(z_sb[:], rz[:, 512:1024], AF.Sigmoid)
                    u = wpool.tile([Bc, 512], f32, tag="u")
                    nc.vector.tensor_mul(u[:], r_sb[:], C[:])
                    w = wpool.tile([Bc, 512], f32, tag="w")
                    nc.vector.tensor_add(w[:], u[:], D[:])
                    n = wpool.tile([Bc, 512], bf16, tag="n")
                    nc.scalar.activation(n[:], w[:], AF.Tanh)
                    d1 = wpool.tile([Bc, 512], bf16, tag="d1")
                    d2 = wpool.tile([Bc, 512], bf16, tag="d2")
                    nc.vector.tensor_sub(d1[:], h_bm[:], n[:])
                    nc.vector.tensor_mul(d2[:], z_sb[:], d1[:])
                    nc.vector.tensor_add(h_bm[:], n[:], d2[:])

                # (e) scan2: x_{s-1}
                if s >= 1:
                    F = p_f.tile([Bc, HH], f32, tag="F")
                    mm(F[:], xtb[0:2, :], wax_t[:], start=True, stop=False)
                    mm(F[:], sv8[0:3, co: co + Bc], wav_t[:],
                       start=False, stop=True)
                    hid = wpool.tile([Bc, HH], bf16, tag="hid")
                    nc.scalar.activation(hid[:], F[:], AF.Relu)
                    scr = wpool.tile([Bc, HH], bf16, tag="scr")
                    dt_ = wpool.tile([Bc, 1], f32, tag="dt")
                    nc.gpsimd.tensor_mul(scr[:], hid[:], wdb_t[:])
                    nc.vector.tensor_reduce(dt_[:], scr[:],
                                            axis=mybir.AxisListType.X,
                                            op=OP.add)
                    aw0 = wpool.tile([Bc, 1], f32, tag="aw0")
                    nc.scalar.activation(aw0[:], dt_[:], AF.Sigmoid,
                                         bias=dbias_t[:])
                    ecp = wpool.tile([Bc, NE], f32, tag="ecp")
                    nc.scalar.copy(ecp[:], E[:])
                    t2 = wpool.tile([Bc, 2], f32, tag="t2")
                    nc.scalar.activation(t2[:], ecp[:, 0:2], AF.Copy,
                                         scale=aw0[:])
                    nc.gpsimd.tensor_add(t2[:], t2[:], ecp[:, 2:4])
                    nc.gpsimd.tensor_add(xstage[:, 0:2], xstage[:, 0:2],
                                         t2[:])
                    nc.scalar.copy(out_buf[:, 2 * (s - 1): 2 * s],
                                   xstage[:, 0:2])
                    nc.vector.transpose(xtf[:], xstage[:])
                    nc.vector.tensor_copy(xtb[0:2, :], xtf[0:2, :])

            nc.sync.dma_start(out=d_out[:], in_=out_buf[:])

    nc.compile()
    return nc


# ------------------------------------------------------------------ interface

def kernel(X0, V, W_ih, W_hh, b_ih, b_hh, Wa1, ba1, Wa2, ba2, Wr, br,
           _trace=False, _tmpdir=None):
    from concourse.bass_utils import run_bass_kernel_spmd

    t_steps = T
    if t_steps not in _PROG_CACHE:
        _PROG_CACHE[t_steps] = _build_program(t_steps)
    nc = _PROG_CACHE[t_steps]

    consts = _prep_consts(W_ih, W_hh, b_ih, b_hh, Wa1, ba1, Wa2, ba2, Wr, br)
    in_maps = []
    for c in range(NCORES):
        core = _prep_core(c, X0, V)
        in_maps.append({**consts, **core})

    res = run_bass_kernel_spmd(nc, in_maps, list(range(NCORES)),
                               trace=_trace, tmpdir=_tmpdir)
    outs = [res.results[c]["out"].reshape(Bc, T, OUT) for c in range(NCORES)]
    out = np.concatenate(outs, axis=0).astype(np.float32)
    if _trace:
        return out, res
    return out



# revision 2
# speedup vs baseline: 1.0246x; 1.0246x over previous
"""AttentionResidualGRU fused Trainium2 kernel — feature-major, v2.

Data parallel over batch (8 cores x 32 rows). All state kept feature-major
(partition = hidden/gate dim, free = batch), so the recurrent matmuls are
weight-stationary: per step 12 gate M-tiles x 4 h K-tiles of
(LDWEIGHTS [128,128]bf16 + MATMUL N=32) which the PE sustains at ~27-35ns
per pair, and h_{t+1} comes out of the gate math already in rhs layout for
the next step — no PE transposes anywhere.

Input projections gx(+biases) are folded into the same PSUM accumulation as
12 extra K=3 matmuls per step against the [3, 32T] v-buffer (rows
[1, v0, v1]); only the first matmul of each bank carries start=True so
per-element has_written semantics make every region overwrite-then-
accumulate correctly.

PSUM banks (8): rA x2 = r pre-acts, zA x2 = z pre-acts, B x2 = gh_n + b_hn,
gN x1 = gx_n, HLE x1 = attention hidden | logit rows [ld,-ld] | E rows
[res0,res1|v0,v1]. Gate tail uses oz = sigmoid(-z_pre) = 1-z so
h' = (h - oz*h) + oz*n. Scan2's elementwise tail runs on GpSimd (vs an
E-copy staged through ACT) to keep the DVE FIFO clear for the GRU chain;
x history lives in the bf16 output buffer and doubles as the attention rhs.
"""

import os
import sys

import numpy as np
import ml_dtypes

BF16 = ml_dtypes.bfloat16

for _p in ("/opt/trn_rl_repo", "/root/.axon_site/_ro/trn_rl_repo"):
    if os.path.isdir(_p) and _p not in sys.path:
        sys.path.append(_p)

B, T, H, IN, OUT = 256, 1024, 512, 2, 2
NCORES = 8
Bc = B // NCORES          # 32
SV_COLS = Bc * T          # 32768

_PROG_CACHE = {}


# ----------------------------------------------------------------- host prep

def _prep_consts(W_ih, W_hh, b_ih, b_hh, Wa1, ba1, Wa2, ba2, Wr, br):
    f = np.float32
    W_ih = np.asarray(W_ih, f); W_hh = np.asarray(W_hh, f)
    b_ih = np.asarray(b_ih, f); b_hh = np.asarray(b_hh, f)
    Wa1 = np.asarray(Wa1, f); ba1 = np.asarray(ba1, f)
    Wa2 = np.asarray(Wa2, f); ba2 = np.asarray(ba2, f)
    Wr = np.asarray(Wr, f); br = np.asarray(br, f)

    def gate_row0(m):
        return 128 * m if m < 4 else (512 + 128 * (m - 4) if m < 8
                                      else 1024 + 128 * (m - 8))

    # 48 stationary gate tiles: tile (m, k) at cols 128*(4m+k).
    wt = np.zeros((128, 48 * 128), f)
    for m in range(12):
        r0 = gate_row0(m)
        for k in range(4):
            wt[:, 128 * (4 * m + k):128 * (4 * m + k + 1)] = \
                W_hh[r0:r0 + 128, 128 * k:128 * (k + 1)].T

    b4 = np.zeros((4, 128), f)       # b_hh n-part, chunk k in row k
    for k in range(4):
        b4[k] = b_hh[1024 + 128 * k:1024 + 128 * (k + 1)]
    sel4 = np.zeros((4, 128), f)     # chunk selector rhs
    for c in range(4):
        sel4[c, 32 * c:32 * (c + 1)] = 1.0

    # in-loop gx stationaries: [3, 128] per chunk c (rows [bias, W0, W1])
    wx = np.zeros((3, 12 * 128), f)
    for c in range(12):
        r0 = gate_row0(c)
        bias = b_ih[r0:r0 + 128] + (b_hh[r0:r0 + 128] if c < 8 else 0.0)
        wx[0, 128 * c:128 * (c + 1)] = bias
        wx[1, 128 * c:128 * (c + 1)] = W_ih[r0:r0 + 128, 0]
        wx[2, 128 * c:128 * (c + 1)] = W_ih[r0:r0 + 128, 1]

    # E matmuls: E2 = [res0, res1 | v0, v1]
    ew = np.zeros((128, 8), f)       # k-tile k at cols 2k: [Wr0, Wr1]
    for k in range(4):
        ew[:, 2 * k + 0] = Wr[0, 128 * k:128 * (k + 1)]
        ew[:, 2 * k + 1] = Wr[1, 128 * k:128 * (k + 1)]
    eaP = np.zeros((3, 2), f)        # over [1, v0, v1]: br row
    eaP[0] = [br[0], br[1]]
    eaQ = np.zeros((3, 2), f)        # [v0, v1]
    eaQ[1, 0] = 1.0
    eaQ[2, 1] = 1.0

    # attention MLP split: v-part [3, 128]x2 (rows [ba1, Wa1v0, Wa1v1]),
    # x-part [2, 128]x2 (rows [Wa1x0, Wa1x1])
    wa1v = np.zeros((3, 256), f)
    wa1x = np.zeros((2, 256), f)
    for j in range(2):
        sl = slice(128 * j, 128 * (j + 1))
        wa1v[0, sl] = ba1[sl]
        wa1v[1, sl] = Wa1[sl, 2]
        wa1v[2, sl] = Wa1[sl, 3]
        wa1x[0, sl] = Wa1[sl, 0]
        wa1x[1, sl] = Wa1[sl, 1]
    wd = (Wa2[0] - Wa2[1]).astype(f)  # [256]
    wd2 = np.zeros((128, 4), f)       # k-tile k at cols 2k: [wd, -wd]
    for k in range(2):
        wd2[:, 2 * k + 0] = wd[128 * k:128 * (k + 1)]
        wd2[:, 2 * k + 1] = -wd[128 * k:128 * (k + 1)]
    db = float(ba2[0] - ba2[1])
    dbias = np.array([[db], [-db]], f)

    return dict(wt=wt.astype(BF16),
                b4=b4.astype(BF16), sel4=sel4.astype(BF16),
                wx=wx.astype(BF16), ew=ew.astype(BF16),
                eaP=eaP.astype(BF16), eaQ=eaQ.astype(BF16),
                wa1v=wa1v.astype(BF16), wa1x=wa1x.astype(BF16),
                wd2=wd2.astype(BF16), dbias=dbias)


def _prep_core(c, X0, V):
    f = np.float32
    bs = slice(Bc * c, Bc * (c + 1))
    Vc = np.asarray(V[bs], f)                      # [32, T, 2]
    sv3 = np.zeros((3, SV_COLS), f)                # rows [1, v0, v1], col 32t+b
    sv3[0] = 1.0
    sv3[1] = Vc[:, :, 0].T.reshape(-1)
    sv3[2] = Vc[:, :, 1].T.reshape(-1)
    xf0 = np.asarray(X0[bs], f).T.copy()           # [2, 32] f32
    return dict(sv3=sv3.astype(BF16), xf0=xf0, xb0=xf0.astype(BF16))


# ------------------------------------------------------------- device program

def _build_program():
    from concourse import bacc, tile, mybir  # noqa

    f32 = mybir.dt.float32
    bf16 = mybir.dt.bfloat16
    AF = mybir.ActivationFunctionType

    nc = bacc.Bacc(None)
    d_wt = nc.declare_dram_parameter("wt", [128, 48 * 128], bf16, isOutput=False)
    d_b4 = nc.declare_dram_parameter("b4", [4, 128], bf16, isOutput=False)
    d_sel = nc.declare_dram_parameter("sel4", [4, 128], bf16, isOutput=False)
    d_wx = nc.declare_dram_parameter("wx", [3, 12 * 128], bf16, isOutput=False)
    d_ew = nc.declare_dram_parameter("ew", [128, 8], bf16, isOutput=False)
    d_eaP = nc.declare_dram_parameter("eaP", [3, 2], bf16, isOutput=False)
    d_eaQ = nc.declare_dram_parameter("eaQ", [3, 2], bf16, isOutput=False)
    d_wa1v = nc.declare_dram_parameter("wa1v", [3, 256], bf16, isOutput=False)
    d_wa1x = nc.declare_dram_parameter("wa1x", [2, 256], bf16, isOutput=False)
    d_wd2 = nc.declare_dram_parameter("wd2", [128, 4], bf16, isOutput=False)
    d_db = nc.declare_dram_parameter("dbias", [2, 1], f32, isOutput=False)
    d_sv = nc.declare_dram_parameter("sv3", [3, SV_COLS], bf16, isOutput=False)
    d_xf0 = nc.declare_dram_parameter("xf0", [2, Bc], f32, isOutput=False)
    d_xb0 = nc.declare_dram_parameter("xb0", [2, Bc], bf16, isOutput=False)
    d_out = nc.declare_dram_parameter("out", [2, T * Bc], bf16, isOutput=True)

    with tile.TileContext(nc) as tc:
        with (
            tc.tile_pool(name="const", bufs=1) as cpool,
            tc.tile_pool(name="state", bufs=1) as spool,
            tc.tile_pool(name="hpool", bufs=2) as hpool,
            tc.tile_pool(name="work", bufs=2) as wpool,
            tc.tile_pool(name="pR", bufs=2, space="PSUM") as pR,
            tc.tile_pool(name="pZ", bufs=2, space="PSUM") as pZ,
            tc.tile_pool(name="pB", bufs=2, space="PSUM") as pB,
            tc.tile_pool(name="pG", bufs=1, space="PSUM") as pG,
            tc.tile_pool(name="pHL", bufs=1, space="PSUM") as pHL,
        ):
            # ---- constants
            wt = cpool.tile([128, 48 * 128], bf16, tag="wt")
            b4_t = cpool.tile([4, 128], bf16, tag="b4")
            sel_t = cpool.tile([4, 128], bf16, tag="sel4")
            wx_t = cpool.tile([3, 12 * 128], bf16, tag="wx")
            ew_t = cpool.tile([128, 8], bf16, tag="ew")
            eaP_t = cpool.tile([3, 2], bf16, tag="eaP")
            eaQ_t = cpool.tile([3, 2], bf16, tag="eaQ")
            wa1v_t = cpool.tile([3, 256], bf16, tag="wa1v")
            wa1x_t = cpool.tile([2, 256], bf16, tag="wa1x")
            wd2_t = cpool.tile([128, 4], bf16, tag="wd2")
            db_t = cpool.tile([2, 1], f32, tag="dbias")
            sv_t = cpool.tile([3, SV_COLS], bf16, tag="sv3")
            for dst, src in ((wt, d_wt), (b4_t, d_b4),
                             (sel_t, d_sel), (wx_t, d_wx), (ew_t, d_ew),
                             (eaP_t, d_eaP), (eaQ_t, d_eaQ), (wa1v_t, d_wa1v),
                             (wa1x_t, d_wa1x), (wd2_t, d_wd2), (db_t, d_db),
                             (sv_t, d_sv)):
                nc.sync.dma_start(out=dst[:], in_=src[:])

            # ---- state
            xf = spool.tile([2, Bc], f32, tag="xf")
            outb = spool.tile([2, T * Bc], bf16, tag="outb")
            xb0 = spool.tile([2, Bc], bf16, tag="xb0")
            nc.sync.dma_start(out=xf[:], in_=d_xf0[:])
            nc.sync.dma_start(out=xb0[:], in_=d_xb0[:])

            mm = nc.tensor.matmul

            h = hpool.tile([128, 128], bf16, tag="h")
            nc.vector.memset(h[:], 0.0)

            for s in range(T + 1):
                last = s == T
                svs = sv_t[:, Bc * s:Bc * (s + 1)] if not last else None
                svp = sv_t[:, Bc * (s - 1):Bc * s]  # v_{s-1} (s>=1)
                h_prev = h

                # 1) h-independent PE work in the prev-step tail
                if not last:
                    rA = pR.tile([128, 128], f32, tag="rA")
                    zA = pZ.tile([128, 128], f32, tag="zA")
                    gN = pG.tile([128, 128], f32, tag="gN")
                    bankB = pB.tile([128, 128], f32, tag="B")
                    for m in range(12):
                        dstA = (rA[:, 32 * m:32 * (m + 1)] if m < 4 else
                                zA[:, 32 * (m - 4):32 * (m - 3)] if m < 8 else
                                gN[:, 32 * (m - 8):32 * (m - 7)])
                        mm(dstA, wx_t[:, 128 * m:128 * (m + 1)], svs,
                           start=(m % 4 == 0), stop=(m >= 8))
                    mm(bankB[:], b4_t[:], sel_t[:], start=True, stop=False)
                if s >= 1:
                    hl = pHL.tile([128, 160], f32, tag="HL")
                    for j in range(2):
                        mm(hl[:, 32 * j:32 * (j + 1)],
                           wa1v_t[:, 128 * j:128 * (j + 1)], svp,
                           start=True, stop=False)

                # 2) n-part and r-part recurrent pairs + gate head
                if not last:
                    for m in (8, 9, 10, 11, 0, 1, 2, 3):
                        dst = (rA[:, 32 * m:32 * (m + 1)] if m < 4
                               else bankB[:, 32 * (m - 8):32 * (m - 7)])
                        for k in range(4):
                            mm(dst,
                               wt[:, 128 * (4 * m + k):128 * (4 * m + k + 1)],
                               h[:, 32 * k:32 * (k + 1)],
                               start=False, stop=(k == 3))
                    rz = wpool.tile([128, 256], bf16, tag="rz")
                    nc.scalar.activation(rz[:, 0:128], rA[:], AF.Sigmoid)
                    u = wpool.tile([128, 128], f32, tag="u")
                    nc.vector.tensor_mul(u[:], rz[:, 0:128], bankB[:])
                    nc.vector.tensor_add(u[:], u[:], gN[:])

                # 3) scan2 head: x-part, relu, logit, aw
                if s >= 1:
                    xprev = (xb0[:] if s == 1
                             else outb[:, Bc * (s - 2):Bc * (s - 1)])
                    for j in range(2):
                        mm(hl[:, 32 * j:32 * (j + 1)],
                           wa1x_t[:, 128 * j:128 * (j + 1)], xprev,
                           start=False, stop=True)
                    hid = wpool.tile([128, 64], bf16, tag="hid")
                    nc.scalar.activation(hid[:], hl[:, 0:64], AF.Relu)
                    for k in range(2):
                        mm(hl[0:2, 64:96], wd2_t[:, 2 * k:2 * (k + 1)],
                           hid[:, 32 * k:32 * (k + 1)],
                           start=(k == 0), stop=(k == 1))
                    aw = wpool.tile([2, Bc], f32, tag="aw")
                    nc.scalar.activation(aw[:], hl[0:2, 64:96], AF.Sigmoid,
                                         bias=db_t[:])

                # 4) E rows from h_s + stage to SBUF
                if s >= 1:
                    for k in range(4):
                        mm(hl[0:2, 96:96 + Bc], ew_t[:, 2 * k:2 * (k + 1)],
                           h_prev[:, 32 * k:32 * (k + 1)],
                           start=(k == 0), stop=False)
                    mm(hl[0:2, 96:96 + Bc], eaP_t[:], svp,
                       start=False, stop=True)
                    mm(hl[0:2, 96 + Bc:96 + 2 * Bc], eaQ_t[:], svp,
                       start=True, stop=True)
                    e2s = wpool.tile([2, 2 * Bc], f32, tag="e2s")
                    nc.scalar.copy(e2s[:], hl[0:2, 96:96 + 2 * Bc])

                # 5) z-part pairs + gate tail
                if not last:
                    for m in (4, 5, 6, 7):
                        for k in range(4):
                            mm(zA[:, 32 * (m - 4):32 * (m - 3)],
                               wt[:, 128 * (4 * m + k):128 * (4 * m + k + 1)],
                               h[:, 32 * k:32 * (k + 1)],
                               start=False, stop=(k == 3))
                    nc.scalar.activation(rz[:, 128:256], zA[:],
                                         AF.Sigmoid, scale=-1.0)
                    n_sb = wpool.tile([128, 128], bf16, tag="n_sb")
                    nc.scalar.activation(n_sb[:], u[:], AF.Tanh)
                    q = wpool.tile([128, 128], bf16, tag="q")
                    nc.vector.tensor_mul(q[:], rz[:, 128:256], h[:])
                    nc.vector.tensor_sub(q[:], h[:], q[:])
                    nc.vector.tensor_mul(n_sb[:], rz[:, 128:256], n_sb[:])
                    h = hpool.tile([128, 128], bf16, tag="h")
                    nc.vector.tensor_add(h[:], q[:], n_sb[:])

                # 6) scan2 tail on GpSimd
                if s >= 1:
                    t01 = wpool.tile([2, Bc], f32, tag="t01")
                    nc.gpsimd.tensor_mul(t01[:], aw[:], e2s[:, 0:Bc])
                    nc.gpsimd.tensor_add(t01[:], t01[:], e2s[:, Bc:2 * Bc])
                    nc.gpsimd.tensor_add(xf[:], xf[:], t01[:])
                    nc.gpsimd.tensor_copy(outb[:, Bc * (s - 1):Bc * s], xf[:])

            nc.sync.dma_start(out=d_out[:], in_=outb[:])

    nc.compile()
    return nc


# ------------------------------------------------------------------ interface

def kernel(X0, V, W_ih, W_hh, b_ih, b_hh, Wa1, ba1, Wa2, ba2, Wr, br,
           _trace=False, _tmpdir=None):
    from concourse.bass_utils import run_bass_kernel_spmd

    if "prog" not in _PROG_CACHE:
        _PROG_CACHE["prog"] = _build_program()
    nc = _PROG_CACHE["prog"]

    consts = _prep_consts(W_ih, W_hh, b_ih, b_hh, Wa1, ba1, Wa2, ba2, Wr, br)
    in_maps = []
    for c in range(NCORES):
        core = _prep_core(c, X0, V)
        in_maps.append({**consts, **core})

    res = run_bass_kernel_spmd(nc, in_maps, list(range(NCORES)),
                               trace=_trace, tmpdir=_tmpdir)
    outs = []
    for c in range(NCORES):
        buf = np.asarray(res.results[c]["out"], dtype=np.float32)  # [2, T*Bc]
        outs.append(buf.reshape(2, T, Bc).transpose(2, 1, 0))
    out = np.concatenate(outs, axis=0)
    if _trace:
        return out, res
    return out


# revision 3
# speedup vs baseline: 1.0351x; 1.0102x over previous
"""AttentionResidualGRU fused Trainium2 kernel — feature-major, v2.

Data parallel over batch (8 cores x 32 rows). All state kept feature-major
(partition = hidden/gate dim, free = batch), so the recurrent matmuls are
weight-stationary: per step 12 gate M-tiles x 4 h K-tiles of
(LDWEIGHTS [128,128]bf16 + MATMUL N=32) which the PE sustains at ~27-35ns
per pair, and h_{t+1} comes out of the gate math already in rhs layout for
the next step — no PE transposes anywhere.

Input projections gx(+biases) are folded into the same PSUM accumulation as
12 extra K=3 matmuls per step against the [3, 32T] v-buffer (rows
[1, v0, v1]); only the first matmul of each bank carries start=True so
per-element has_written semantics make every region overwrite-then-
accumulate correctly.

PSUM banks (8): rA x2 = r pre-acts, zA x2 = z pre-acts, B x2 = gh_n + b_hn,
gN x1 = gx_n, HLE x1 = attention hidden | logit rows [ld,-ld] | E rows
[res0,res1|v0,v1]. Gate tail uses oz = sigmoid(-z_pre) = 1-z so
h' = (h - oz*h) + oz*n. Scan2's elementwise tail runs on GpSimd (vs an
E-copy staged through ACT) to keep the DVE FIFO clear for the GRU chain;
x history lives in the bf16 output buffer and doubles as the attention rhs.
"""

import os
import sys

import numpy as np
import ml_dtypes

BF16 = ml_dtypes.bfloat16

for _p in ("/opt/trn_rl_repo", "/root/.axon_site/_ro/trn_rl_repo"):
    if os.path.isdir(_p) and _p not in sys.path:
        sys.path.append(_p)

B, T, H, IN, OUT = 256, 1024, 512, 2, 2
NCORES = 8
Bc = B // NCORES          # 32
SV_COLS = Bc * T          # 32768

_PROG_CACHE = {}


# ----------------------------------------------------------------- host prep

def _prep_consts(W_ih, W_hh, b_ih, b_hh, Wa1, ba1, Wa2, ba2, Wr, br):
    f = np.float32
    W_ih = np.asarray(W_ih, f); W_hh = np.asarray(W_hh, f)
    b_ih = np.asarray(b_ih, f); b_hh = np.asarray(b_hh, f)
    Wa1 = np.asarray(Wa1, f); ba1 = np.asarray(ba1, f)
    Wa2 = np.asarray(Wa2, f); ba2 = np.asarray(ba2, f)
    Wr = np.asarray(Wr, f); br = np.asarray(br, f)

    def gate_row0(m):
        return 128 * m if m < 4 else (512 + 128 * (m - 4) if m < 8
                                      else 1024 + 128 * (m - 8))

    # 48 stationary gate tiles: tile (m, k) at cols 128*(4m+k).
    wt = np.zeros((128, 48 * 128), f)
    for m in range(12):
        r0 = gate_row0(m)
        for k in range(4):
            wt[:, 128 * (4 * m + k):128 * (4 * m + k + 1)] = \
                W_hh[r0:r0 + 128, 128 * k:128 * (k + 1)].T

    b4 = np.zeros((4, 128), f)       # b_hh n-part, chunk k in row k
    for k in range(4):
        b4[k] = b_hh[1024 + 128 * k:1024 + 128 * (k + 1)]
    sel4 = np.zeros((4, 128), f)     # chunk selector rhs
    for c in range(4):
        sel4[c, 32 * c:32 * (c + 1)] = 1.0

    # in-loop gx stationaries: [3, 128] per chunk c (rows [bias, W0, W1])
    wx = np.zeros((3, 12 * 128), f)
    for c in range(12):
        r0 = gate_row0(c)
        bias = b_ih[r0:r0 + 128] + (b_hh[r0:r0 + 128] if c < 8 else 0.0)
        wx[0, 128 * c:128 * (c + 1)] = bias
        wx[1, 128 * c:128 * (c + 1)] = W_ih[r0:r0 + 128, 0]
        wx[2, 128 * c:128 * (c + 1)] = W_ih[r0:r0 + 128, 1]

    # E matmuls: E2 = [res0, res1 | v0, v1]
    ew = np.zeros((128, 8), f)       # k-tile k at cols 2k: [Wr0, Wr1]
    for k in range(4):
        ew[:, 2 * k + 0] = Wr[0, 128 * k:128 * (k + 1)]
        ew[:, 2 * k + 1] = Wr[1, 128 * k:128 * (k + 1)]
    eaP = np.zeros((3, 2), f)        # over [1, v0, v1]: br row
    eaP[0] = [br[0], br[1]]
    eaQ = np.zeros((3, 2), f)        # [v0, v1]
    eaQ[1, 0] = 1.0
    eaQ[2, 1] = 1.0

    # attention MLP split: v-part [3, 128]x2 (rows [ba1, Wa1v0, Wa1v1]),
    # x-part [2, 128]x2 (rows [Wa1x0, Wa1x1])
    wa1v = np.zeros((3, 256), f)
    wa1x = np.zeros((2, 256), f)
    for j in range(2):
        sl = slice(128 * j, 128 * (j + 1))
        wa1v[0, sl] = ba1[sl]
        wa1v[1, sl] = Wa1[sl, 2]
        wa1v[2, sl] = Wa1[sl, 3]
        wa1x[0, sl] = Wa1[sl, 0]
        wa1x[1, sl] = Wa1[sl, 1]
    wd = (Wa2[0] - Wa2[1]).astype(f)  # [256]
    wd2 = np.zeros((128, 4), f)       # k-tile k at cols 2k: [wd, -wd]
    for k in range(2):
        wd2[:, 2 * k + 0] = wd[128 * k:128 * (k + 1)]
        wd2[:, 2 * k + 1] = -wd[128 * k:128 * (k + 1)]
    db = float(ba2[0] - ba2[1])
    dbias = np.array([[db], [-db]], f)

    return dict(wt=wt.astype(BF16),
                b4=b4.astype(BF16), sel4=sel4.astype(BF16),
                wx=wx.astype(BF16), ew=ew.astype(BF16),
                eaP=eaP.astype(BF16), eaQ=eaQ.astype(BF16),
                wa1v=wa1v.astype(BF16), wa1x=wa1x.astype(BF16),
                wd2=wd2.astype(BF16), dbias=dbias)


def _prep_core(c, X0, V):
    f = np.float32
    bs = slice(Bc * c, Bc * (c + 1))
    Vc = np.asarray(V[bs], f)                      # [32, T, 2]
    sv3 = np.zeros((3, SV_COLS), f)                # rows [1, v0, v1], col 32t+b
    sv3[0] = 1.0
    sv3[1] = Vc[:, :, 0].T.reshape(-1)
    sv3[2] = Vc[:, :, 1].T.reshape(-1)
    xf0 = np.asarray(X0[bs], f).T.copy()           # [2, 32] f32
    return dict(sv3=sv3.astype(BF16), xf0=xf0, xb0=xf0.astype(BF16))


# ------------------------------------------------------------- device program

def _build_program():
    from concourse import bacc, tile, mybir  # noqa

    f32 = mybir.dt.float32
    bf16 = mybir.dt.bfloat16
    AF = mybir.ActivationFunctionType

    nc = bacc.Bacc(None)
    d_wt = nc.declare_dram_parameter("wt", [128, 48 * 128], bf16, isOutput=False)
    d_b4 = nc.declare_dram_parameter("b4", [4, 128], bf16, isOutput=False)
    d_sel = nc.declare_dram_parameter("sel4", [4, 128], bf16, isOutput=False)
    d_wx = nc.declare_dram_parameter("wx", [3, 12 * 128], bf16, isOutput=False)
    d_ew = nc.declare_dram_parameter("ew", [128, 8], bf16, isOutput=False)
    d_eaP = nc.declare_dram_parameter("eaP", [3, 2], bf16, isOutput=False)
    d_eaQ = nc.declare_dram_parameter("eaQ", [3, 2], bf16, isOutput=False)
    d_wa1v = nc.declare_dram_parameter("wa1v", [3, 256], bf16, isOutput=False)
    d_wa1x = nc.declare_dram_parameter("wa1x", [2, 256], bf16, isOutput=False)
    d_wd2 = nc.declare_dram_parameter("wd2", [128, 4], bf16, isOutput=False)
    d_db = nc.declare_dram_parameter("dbias", [2, 1], f32, isOutput=False)
    d_sv = nc.declare_dram_parameter("sv3", [3, SV_COLS], bf16, isOutput=False)
    d_xf0 = nc.declare_dram_parameter("xf0", [2, Bc], f32, isOutput=False)
    d_xb0 = nc.declare_dram_parameter("xb0", [2, Bc], bf16, isOutput=False)
    d_out = nc.declare_dram_parameter("out", [2, T * Bc], bf16, isOutput=True)

    with tile.TileContext(nc) as tc:
        with (
            tc.tile_pool(name="const", bufs=1) as cpool,
            tc.tile_pool(name="state", bufs=1) as spool,
            tc.tile_pool(name="hpool", bufs=2) as hpool,
            tc.tile_pool(name="work", bufs=2) as wpool,
            tc.tile_pool(name="pR", bufs=2, space="PSUM") as pR,
            tc.tile_pool(name="pZ", bufs=2, space="PSUM") as pZ,
            tc.tile_pool(name="pB", bufs=2, space="PSUM") as pB,
            tc.tile_pool(name="pG", bufs=1, space="PSUM") as pG,
            tc.tile_pool(name="pHL", bufs=1, space="PSUM") as pHL,
        ):
            # ---- constants
            wt = cpool.tile([128, 48 * 128], bf16, tag="wt")
            b4_t = cpool.tile([4, 128], bf16, tag="b4")
            sel_t = cpool.tile([4, 128], bf16, tag="sel4")
            wx_t = cpool.tile([3, 12 * 128], bf16, tag="wx")
            ew_t = cpool.tile([128, 8], bf16, tag="ew")
            eaP_t = cpool.tile([3, 2], bf16, tag="eaP")
            eaQ_t = cpool.tile([3, 2], bf16, tag="eaQ")
            wa1v_t = cpool.tile([3, 256], bf16, tag="wa1v")
            wa1x_t = cpool.tile([2, 256], bf16, tag="wa1x")
            wd2_t = cpool.tile([128, 4], bf16, tag="wd2")
            db_t = cpool.tile([2, 1], f32, tag="dbias")
            sv_t = cpool.tile([3, SV_COLS], bf16, tag="sv3")
            for dst, src in ((wt, d_wt), (b4_t, d_b4),
                             (sel_t, d_sel), (wx_t, d_wx), (ew_t, d_ew),
                             (eaP_t, d_eaP), (eaQ_t, d_eaQ), (wa1v_t, d_wa1v),
                             (wa1x_t, d_wa1x), (wd2_t, d_wd2), (db_t, d_db),
                             (sv_t, d_sv)):
                nc.sync.dma_start(out=dst[:], in_=src[:])

            # ---- state
            xf = spool.tile([2, Bc], f32, tag="xf")
            outb = spool.tile([2, T * Bc], bf16, tag="outb")
            xb0 = spool.tile([2, Bc], bf16, tag="xb0")
            nc.sync.dma_start(out=xf[:], in_=d_xf0[:])
            nc.sync.dma_start(out=xb0[:], in_=d_xb0[:])

            mm = nc.tensor.matmul

            h = hpool.tile([128, 128], bf16, tag="h")
            nc.vector.memset(h[:], 0.0)

            for s in range(T + 1):
                last = s == T
                svs = sv_t[:, Bc * s:Bc * (s + 1)] if not last else None
                svp = sv_t[:, Bc * (s - 1):Bc * s]  # v_{s-1} (s>=1)
                h_prev = h

                # 1) h-independent PE work in the prev-step tail
                if not last:
                    rA = pR.tile([128, 128], f32, tag="rA")
                    zA = pZ.tile([128, 128], f32, tag="zA")
                    gN = pG.tile([128, 128], f32, tag="gN")
                    bankB = pB.tile([128, 128], f32, tag="B")
                    for m in range(12):
                        dstA = (rA[:, 32 * m:32 * (m + 1)] if m < 4 else
                                zA[:, 32 * (m - 4):32 * (m - 3)] if m < 8 else
                                gN[:, 32 * (m - 8):32 * (m - 7)])
                        mm(dstA, wx_t[:, 128 * m:128 * (m + 1)], svs,
                           start=(m % 4 == 0), stop=(m >= 8))
                    mm(bankB[:], b4_t[:], sel_t[:], start=True, stop=False)
                if s >= 1:
                    hl = pHL.tile([128, 160], f32, tag="HL")
                    for j in range(2):
                        mm(hl[:, 32 * j:32 * (j + 1)],
                           wa1v_t[:, 128 * j:128 * (j + 1)], svp,
                           start=True, stop=False)

                # 2) n-part and r-part recurrent pairs + gate head
                if not last:
                    for m in (8, 9, 10, 11, 0, 1, 2, 3):
                        dst = (rA[:, 32 * m:32 * (m + 1)] if m < 4
                               else bankB[:, 32 * (m - 8):32 * (m - 7)])
                        for k in range(4):
                            mm(dst,
                               wt[:, 128 * (4 * m + k):128 * (4 * m + k + 1)],
                               h[:, 32 * k:32 * (k + 1)],
                               start=False, stop=(k == 3))
                    rz = wpool.tile([128, 256], bf16, tag="rz")
                    nc.scalar.activation(rz[:, 0:128], rA[:], AF.Sigmoid)
                    u = wpool.tile([128, 128], f32, tag="u")
                    nc.vector.tensor_mul(u[:], rz[:, 0:128], bankB[:])
                    nc.vector.tensor_add(u[:], u[:], gN[:])

                # 3) scan2 head: x-part, relu, logit, aw
                if s >= 1:
                    xprev = (xb0[:] if s == 1
                             else outb[:, Bc * (s - 2):Bc * (s - 1)])
                    for j in range(2):
                        mm(hl[:, 32 * j:32 * (j + 1)],
                           wa1x_t[:, 128 * j:128 * (j + 1)], xprev,
                           start=False, stop=True)
                    hid = wpool.tile([128, 64], bf16, tag="hid")
                    nc.scalar.activation(hid[:], hl[:, 0:64], AF.Relu)
                    for k in range(2):
                        mm(hl[0:2, 64:96], wd2_t[:, 2 * k:2 * (k + 1)],
                           hid[:, 32 * k:32 * (k + 1)],
                           start=(k == 0), stop=(k == 1))
                    aw = wpool.tile([2, Bc], f32, tag="aw")
                    nc.scalar.activation(aw[:], hl[0:2, 64:96], AF.Sigmoid,
                                         bias=db_t[:])

                # 4) E rows from h_s + stage to SBUF
                if s >= 1:
                    for k in range(4):
                        mm(hl[0:2, 96:96 + Bc], ew_t[:, 2 * k:2 * (k + 1)],
                           h_prev[:, 32 * k:32 * (k + 1)],
                           start=(k == 0), stop=False)
                    mm(hl[0:2, 96:96 + Bc], eaP_t[:], svp,
                       start=False, stop=True)
                    mm(hl[0:2, 96 + Bc:96 + 2 * Bc], eaQ_t[:], svp,
                       start=True, stop=True)
                    e2s = wpool.tile([2, 2 * Bc], f32, tag="e2s")
                    nc.scalar.copy(e2s[:], hl[0:2, 96:96 + 2 * Bc])

                # 5) z-part pairs + gate tail
                if not last:
                    for m in (4, 5, 6, 7):
                        for k in range(4):
                            mm(zA[:, 32 * (m - 4):32 * (m - 3)],
                               wt[:, 128 * (4 * m + k):128 * (4 * m + k + 1)],
                               h[:, 32 * k:32 * (k + 1)],
                               start=False, stop=(k == 3))
                    nc.scalar.activation(rz[:, 128:256], zA[:],
                                         AF.Sigmoid, scale=-1.0)
                    n_sb = wpool.tile([128, 128], bf16, tag="n_sb")
                    nc.scalar.activation(n_sb[:], u[:], AF.Tanh)
                    q = wpool.tile([128, 128], bf16, tag="q")
                    nc.vector.tensor_mul(q[:], rz[:, 128:256], h[:])
                    nc.vector.tensor_sub(q[:], h[:], q[:])
                    nc.vector.tensor_mul(n_sb[:], rz[:, 128:256], n_sb[:])
                    h = hpool.tile([128, 128], bf16, tag="h")
                    nc.vector.tensor_add(h[:], q[:], n_sb[:])

                # 6) scan2 tail on GpSimd
                if s >= 1:
                    t01 = wpool.tile([2, Bc], f32, tag="t01")
                    nc.vector.tensor_mul(t01[:], aw[:], e2s[:, 0:Bc])
                    nc.vector.tensor_add(t01[:], t01[:], e2s[:, Bc:2 * Bc])
                    nc.vector.tensor_add(xf[:], xf[:], t01[:])
                    nc.vector.tensor_copy(outb[:, Bc * (s - 1):Bc * s], xf[:])

            nc.sync.dma_start(out=d_out[:], in_=outb[:])

    nc.compile()
    return nc


# ------------------------------------------------------------------ interface

def kernel(X0, V, W_ih, W_hh, b_ih, b_hh, Wa1, ba1, Wa2, ba2, Wr, br,
           _trace=False, _tmpdir=None):
    from concourse.bass_utils import run_bass_kernel_spmd

    if "prog" not in _PROG_CACHE:
        _PROG_CACHE["prog"] = _build_program()
    nc = _PROG_CACHE["prog"]

    consts = _prep_consts(W_ih, W_hh, b_ih, b_hh, Wa1, ba1, Wa2, ba2, Wr, br)
    in_maps = []
    for c in range(NCORES):
        core = _prep_core(c, X0, V)
        in_maps.append({**consts, **core})

    res = run_bass_kernel_spmd(nc, in_maps, list(range(NCORES)),
                               trace=_trace, tmpdir=_tmpdir)
    outs = []
    for c in range(NCORES):
        buf = np.asarray(res.results[c]["out"], dtype=np.float32)  # [2, T*Bc]
        outs.append(buf.reshape(2, T, Bc).transpose(2, 1, 0))
    out = np.concatenate(outs, axis=0)
    if _trace:
        return out, res
    return out
